# revision 14
# baseline (speedup 1.0000x reference)
"""Trainium2 Bass kernel for nn_Density_Block (histogram_binning).

Computes, for N=1M rows:
    out1       = softmax(x @ weight + bias, axis=1)        [N, 101]
    out_interp = lerp of two adjacent bins of out1 at t*B   [N]

Strategy (8 NeuronCores, pure data parallel, DMA-bandwidth-bound):
  * Host sorts rows by lower-bin index Li and interleaves sorted rows
    round-robin across cores, so every 10-tile device group (on every
    core) gathers its two softmax bins from one 4-wide column window
    known at program-build time.
  * Matmul: ONE fp16 matmul per 128-row tile (K=66):
        [ones; ones; x_f16] @ [b_hi; b_lo; W_f16]
    accumulated in fp32 PSUM (logit error ~2e-3 abs from fp16 rounding
    of x and W; bias kept fp32-exact via its hi/lo split).  Set
    kernel.TWO_MM=True for the higher-precision two-matmul variant
    (adds x_lo rows, ~1e-4 error, +8MB traffic).
  * Bins padded 101 -> 102: pad logit = -100 (exp ~ 0), the pad column
    carries out_interp, so one DMA per 2 groups writes both outputs.
  * Per 10-tile group (2 PSUM banks): grouped exp on ScalarE; softmax
    sums (segmented reduce) + reciprocal + interp reduce on VectorE;
    normalize + interp window multiply on GpSimd (stride-0 broadcasts).
  * All DRAM I/O partition-major, batched over 2-group supergroups, big
    transfers alternated between the two HWDGE queues (SP / Activation).
"""

import os
import sys
import types
from contextlib import ExitStack

import numpy as np

import concourse.bass as bass
import concourse.tile as tile
from concourse import bacc, mybir
from concourse import bass_utils

F32 = mybir.dt.float32
F16 = mybir.dt.float16

N_CORES = 8
NBINS = 101
NB_PAD = 102
IND = 64
TILE = 128           # rows per tile (SBUF partitions)
TPB = 5              # tiles per PSUM bank (5 * 102 = 510 <= 512 floats)
BPG = 2              # PSUM banks per group
TPG = TPB * BPG      # tiles per group = 10
GROUP_ROWS = TILE * TPG          # 1280
GPS = 2              # groups per supergroup (DMA batching)
SG_ROWS = GROUP_ROWS * GPS
WIN = 4              # gather window width (needs intra-window Li spread <= 2)
PAD_LOGIT = -100.0
BANK = 512           # PSUM bank stride in f32 elements
K1 = IND + 2         # ones(b_hi) + ones(b_lo) + x_hi
TWO_MM = False       # True: add x_lo rows + second matmul (higher precision)

LAST_RESULT = None   # stashed for the local test harness


def _install_ntff_hook():
    try:
        from antenv.axon_hooks import get_axon_ntff_profile_hook  # noqa: F401
        return
    except ImportError:
        pass
    try:
        import antenv
        from trn_agent_boot.trn_boot import _ntff_profile_via_ctypes
        mod = types.ModuleType("antenv.axon_hooks")
        hook = [_ntff_profile_via_ctypes("/opt/axon/libaxon_pjrt.so")]
        mod.set_axon_ntff_profile_hook = lambda h: hook.__setitem__(0, h)
        mod.get_axon_ntff_profile_hook = lambda: hook[0]
        sys.modules["antenv.axon_hooks"] = mod
        antenv.axon_hooks = mod
    except Exception:
        pass


def host_prepare(t, x, weight, bias, num_grid, n_cores=N_CORES):
    """Sort/shard/precompute on host.  Returns (meta, per-core input maps)."""
    t = np.ascontiguousarray(np.asarray(t, dtype=np.float32))
    x = np.asarray(x, dtype=np.float32)
    weight = np.asarray(weight, dtype=np.float32)
    bias = np.asarray(bias, dtype=np.float32)
    B = int(num_grid)
    N = t.shape[0]

    tB = t * np.float32(B)
    U = np.ceil(tB)
    inter = np.float32(1.0) - (U - tB)
    L = U - np.float32(1.0)
    L = np.where(L < 0, L + np.float32(1.0), L)
    Li = L.astype(np.int32)
    Ui = U.astype(np.int32)

    chunk = n_cores * SG_ROWS
    NP = ((N + chunk - 1) // chunk) * chunk
    npad = NP - N
    R = NP // n_cores
    J = R // TILE
    n_windows = J // TPG
    winrows = TILE * n_cores * TPG

    perm = np.argsort(Li, kind="stable")
    Li_s = np.concatenate([Li[perm], np.full(npad, Li[perm[-1]] if N else 0, np.int32)])
    Ui_s = np.concatenate([Ui[perm], np.full(npad, 0, np.int32)])
    inter_s = np.concatenate([inter[perm], np.zeros(npad, np.float32)])

    LO = np.minimum(Li_s[::winrows], NB_PAD - WIN).astype(np.int32)
    assert LO.shape[0] == n_windows
    lo_per_row = np.repeat(LO, winrows)
    spread_ok = (Li_s - lo_per_row >= 0) & (np.maximum(Li_s, Ui_s) - lo_per_row < WIN)
    if not spread_ok[:N].all():
        bad = np.flatnonzero(~spread_ok[:N])[:5]
        raise AssertionError(f"gather-window assumption violated at sorted rows {bad}")

    coef_s = np.zeros((NP, WIN), np.float32)
    rows = np.arange(N)
    np.add.at(coef_s, (rows, (Li_s[:N] - lo_per_row[:N])), np.float32(1.0) - inter_s[:N])
    np.add.at(coef_s, (rows, (Ui_s[:N] - lo_per_row[:N])), inter_s[:N])

    # Weights fp16; bias exact via hi/lo rows.
    wpad = np.zeros((IND, NB_PAD), np.float32)
    wpad[:, :NBINS] = weight
    bpad = np.zeros(NB_PAD, np.float32)
    bpad[:NBINS] = bias
    bpad[NBINS] = np.float32(PAD_LOGIT)
    w_hi = wpad.astype(np.float16)
    b_hi = bpad.astype(np.float16)
    b_lo = (bpad - b_hi.astype(np.float32)).astype(np.float16)
    w1 = np.empty((K1, NB_PAD), np.float16)
    w1[0] = b_hi
    w1[1] = b_lo
    w1[2:] = w_hi
    if TWO_MM:
        w_lo = (wpad - w_hi.astype(np.float32)).astype(np.float16)
        w2 = np.concatenate([w_lo, w_hi[:IND - 1]], 0).astype(np.float16)  # [127,102]

    xs = np.zeros((NP, IND), np.float32)
    xs[:N] = x[perm]
    in_maps = []
    for i in range(n_cores):
        xi = xs[i::n_cores]
        xi_hi = xi.astype(np.float16)
        xst = np.empty((K1 + (IND if TWO_MM else 0), R), np.float16)
        xst[0] = np.float16(1.0)
        xst[1] = np.float16(1.0)
        xst[2:K1] = xi_hi.T
        if TWO_MM:
            xi_lo = (xi - xi_hi.astype(np.float32)).astype(np.float16)
            xst[K1:] = xi_lo.T
        ci = coef_s[i::n_cores].reshape(J, TILE, WIN).transpose(1, 0, 2) \
            .reshape(TILE, J * WIN)
        m = {"xst": xst, "w1": w1, "coef": np.ascontiguousarray(ci)}
        if TWO_MM:
            m["w2"] = w2
        in_maps.append(m)

    meta = dict(N=N, NP=NP, R=R, J=J, LO=LO, perm=perm, n_cores=n_cores)
    return meta, in_maps


def build_program(LO, R, n_cores=N_CORES):
    """Build + compile the (SPMD-identical) Bass program for one core."""
    J = R // TILE
    n_sgroups = R // SG_ROWS
    assert n_sgroups * SG_ROWS == R
    assert len(LO) == J // TPG

    KX = K1 + (IND if TWO_MM else 0)
    nc = bacc.Bacc("TRN2", target_bir_lowering=False, debug=False,
                   num_devices=n_cores)
    xst = nc.dram_tensor("xst", [KX, R], F16, kind="ExternalInput").ap()
    w1 = nc.dram_tensor("w1", [K1, NB_PAD], F16, kind="ExternalInput").ap()
    if TWO_MM:
        w2 = nc.dram_tensor("w2", [IND * 2 - 1, NB_PAD], F16, kind="ExternalInput").ap()
    coef = nc.dram_tensor("coef", [TILE, J * WIN], F32, kind="ExternalInput").ap()
    comb = nc.dram_tensor("comb", [TILE, J * NB_PAD], F32, kind="ExternalOutput").ap()

    Exp = mybir.ActivationFunctionType.Exp
    mult = mybir.AluOpType.mult
    add = mybir.AluOpType.add
    X = mybir.AxisListType.X

    with tile.TileContext(nc) as tc:
        with ExitStack() as ctx:
            wpool = ctx.enter_context(tc.tile_pool(name="w", bufs=1))
            xpool = ctx.enter_context(tc.tile_pool(name="x", bufs=4))
            cpool = ctx.enter_context(tc.tile_pool(name="c", bufs=4))
            ppool = ctx.enter_context(tc.tile_pool(name="ps", bufs=4, space="PSUM"))
            epool = ctx.enter_context(tc.tile_pool(name="ex", bufs=6))
            opool = ctx.enter_context(tc.tile_pool(name="o1", bufs=4))
            spool = ctx.enter_context(tc.tile_pool(name="sm", bufs=8))
            tpool = ctx.enter_context(tc.tile_pool(name="tt", bufs=8))

            w1t = wpool.tile([K1, NB_PAD], F16)
            nc.sync.dma_start(w1t[:], w1[:])
            if TWO_MM:
                w2t = wpool.tile([IND * 2 - 1, NB_PAD], F16)
                nc.sync.dma_start(w2t[:], w2[:])

            for sg in range(n_sgroups):
                qeng = nc.sync if sg % 2 == 0 else nc.scalar
                qeng2 = nc.scalar if sg % 2 == 0 else nc.sync
                c0 = sg * SG_ROWS
                xt = xpool.tile([KX, SG_ROWS], F16)
                qeng.dma_start(xt[:], xst[:, c0:c0 + SG_ROWS])
                cf = cpool.tile([TILE, GPS * TPG * WIN], F32)
                qeng.dma_start(
                    cf[:], coef[:, sg * GPS * TPG * WIN:(sg + 1) * GPS * TPG * WIN])

                o1 = opool.tile([128, GPS * TPG * NB_PAD], F32)
                for h in range(GPS):
                    g = sg * GPS + h
                    ps = ppool.tile([128, BPG * BANK], F32)
                    for ti in range(TPG):
                        o = (ti // TPB) * BANK + (ti % TPB) * NB_PAD
                        xsl = slice((h * TPG + ti) * TILE, (h * TPG + ti + 1) * TILE)
                        if TWO_MM:
                            nc.tensor.matmul(ps[:, o:o + NB_PAD], lhsT=xt[0:K1, xsl],
                                             rhs=w1t[:], start=True, stop=False)
                            nc.tensor.matmul(ps[:, o:o + NB_PAD],
                                             lhsT=xt[2:KX - 1, xsl],
                                             rhs=w2t[:], start=False, stop=True)
                        else:
                            nc.tensor.matmul(ps[:, o:o + NB_PAD], lhsT=xt[:, xsl],
                                             rhs=w1t[:], start=True, stop=True)

                    ex = epool.tile([128, TPG * NB_PAD], F32)
                    nc.scalar.activation(
                        ex[:].rearrange("p (b c) -> p b c", b=BPG),
                        ps[:].rearrange("p (b c) -> p b c", b=BPG)[:, :, 0:TPB * NB_PAD],
                        Exp,
                    )

                    sgt = spool.tile([128, TPG], F32)
                    nc.vector.tensor_reduce(
                        sgt[:], ex[:].rearrange("p (t c) -> p t c", t=TPG),
                        axis=X, op=add,
                    )
                    rg = spool.tile([128, TPG], F32)
                    nc.vector.reciprocal(rg[:], sgt[:])

                    o1h = o1[:, h * TPG * NB_PAD:(h + 1) * TPG * NB_PAD]

                    # interp: window multiply on GpSimd, reduce + 1/s on DVE
                    lo = int(LO[g])
                    tt = tpool.tile([128, TPG * WIN], F32)
                    nc.gpsimd.tensor_tensor(
                        tt[:].rearrange("p (t c) -> p t c", t=TPG),
                        ex[:].rearrange("p (t c) -> p t c", t=TPG)[:, :, lo:lo + WIN],
                        cf[:, h * TPG * WIN:(h + 1) * TPG * WIN]
                          .rearrange("p (t c) -> p t c", t=TPG),
                        op=mult,
                    )
                    ri = spool.tile([128, TPG], F32)
                    nc.vector.tensor_reduce(
                        ri[:], tt[:].rearrange("p (t c) -> p t c", t=TPG),
                        axis=X, op=add,
                    )
                    nc.vector.tensor_tensor(
                        o1h.rearrange("p (t c) -> p t c", t=TPG)[:, :, NBINS:NB_PAD],
                        ri[:].broadcast_to((128, TPG, 1)),
                        rg[:].broadcast_to((128, TPG, 1)),
                        op=mult,
                    )

                    # normalize the real 101 bins on GpSimd
                    nc.gpsimd.tensor_tensor(
                        o1h.rearrange("p (t c) -> p t c", t=TPG)[:, :, 0:NBINS],
                        ex[:].rearrange("p (t c) -> p t c", t=TPG)[:, :, 0:NBINS],
                        rg[:].broadcast_to((128, TPG, NBINS)),
                        op=mult,
                    )

                qeng2.dma_start(
                    comb[:, sg * GPS * TPG * NB_PAD:(sg + 1) * GPS * TPG * NB_PAD],
                    o1[:])

    nc.compile()
    return nc


def kernel(t, x, weight, bias, num_grid):
    global LAST_RESULT
    trace = bool(os.environ.get("BASS_TRACE"))
    if trace:
        _install_ntff_hook()
        bass_utils.upload_artifacts = lambda tmpdir: "local://" + tmpdir

    meta, in_maps = host_prepare(t, x, weight, bias, num_grid)
    nc = build_program(meta["LO"], meta["R"], meta["n_cores"])

    res = bass_utils.run_bass_kernel_spmd(
        nc, in_maps, core_ids=list(range(meta["n_cores"])), trace=trace,
    )
    LAST_RESULT = res

    N, NP, n_cores = meta["N"], meta["NP"], meta["n_cores"]
    R, J = meta["R"], meta["J"]
    perm = meta["perm"]
    comb_s = np.empty((NP, NB_PAD), np.float32)
    for i in range(n_cores):
        ci = res.results[i]["comb"].reshape(TILE, J, NB_PAD)
        comb_s[i::n_cores] = ci.transpose(1, 0, 2).reshape(R, NB_PAD)
    out1 = np.empty((N, NBINS), np.float32)
    oint = np.empty((N,), np.float32)
    out1[perm] = comb_s[:N, :NBINS]
    oint[perm] = comb_s[:N, NBINS]
    return out1, oint
